# revision 14
# baseline (speedup 1.0000x reference)
"""Bahdanau-attention-with-coverage kernel for Trainium2 (Bass/Tile).

The reference module applies softmax over a size-1 axis, so every attention
weight is exactly 1.0. The whole network therefore collapses:
  context_vector     = values.sum(axis=1)               # [B, D]
  attention_weights  = ones([B, T, 1])
  coverage_vector    = arange(T) broadcast to [B, T, 1]
The kernel computes exactly that, data-parallel over batch across 8 cores.
Per core: sum-over-T of a [4, 1024, 1024] f32 slab (DMA-bound, ~16MB read),
plus memset/iota for the trivial outputs.
"""

import numpy as np

import concourse.bacc as bacc
import concourse.mybir as mybir
from concourse.tile import TileContext
from concourse.bass_utils import run_bass_kernel_spmd

B, T, D = 32, 1024, 1024
N_CORES = 8
BL = B // N_CORES  # batches per core
F32 = mybir.dt.float32
P = 128            # SBUF partitions
NCH = T // P       # 8 t-rows per partition

_nc = None


def _build():
    # 4 SWDGE queues: the 3 small output DMAs go out via gpsimd so each gets
    # its own SW-DGE semaphore; the 8 big input DMAs use the 8 HW-DGE queues
    # exactly once each.  (TRN2 codegen allows only ONE sync-wait per
    # instruction; any queue-sem reuse would add a second wait.)
    nc = bacc.Bacc("TRN2", target_bir_lowering=False, num_swdge_queues=4)

    vals = nc.dram_tensor("values", [BL, T, D], F32, kind="ExternalInput")
    ctx_o = nc.dram_tensor("ctx", [BL, D], F32, kind="ExternalOutput")
    aw_o = nc.dram_tensor("aw", [BL, T, 1], F32, kind="ExternalOutput")
    cov_o = nc.dram_tensor("cov", [BL, T, 1], F32, kind="ExternalOutput")

    with TileContext(nc) as tc:
        with (
            # bufs=4 = one slot per batch: no slot reuse within an execution,
            # so no WAW/WAR waits pile up on any instruction (TRN2 codegen
            # allows very few sync-waits per instruction).
            tc.tile_pool(name="big", bufs=4) as big,
            tc.tile_pool(name="mid", bufs=4) as mid,
            tc.tile_pool(name="small", bufs=1) as small,
            tc.tile_pool(name="psum", bufs=4, space="PSUM") as psum,
        ):
            # attention_weights = 1.0 everywhere
            ones_t = small.tile([P, BL * T // P], F32)
            nc.vector.memset(ones_t[:], 1.0)
            nc.gpsimd.dma_start(
                out=aw_o.ap().flatten_outer_dims().rearrange("(p n) c -> p (n c)", p=P),
                in_=ones_t[:],
            )

            # coverage[b, t] = t;  layout t = NCH*p + n  ->  iota = NCH*p + n
            iota_t = small.tile([P, BL, NCH], mybir.dt.int32)
            nc.gpsimd.iota(
                iota_t[:], pattern=[[0, BL], [1, NCH]], base=0, channel_multiplier=NCH
            )
            cov_t = small.tile([P, BL, NCH], F32)
            nc.vector.tensor_copy(out=cov_t[:], in_=iota_t[:])
            nc.gpsimd.dma_start(
                out=cov_o.ap().rearrange("b (p n) c -> p b (n c)", p=P),
                in_=cov_t[:],
            )

            # Sliding-window indicator weights: wz[:, 4] == 1, else 0.  For
            # batch b, lhsT = wz[:, 4-b : 8-b] is [128, BL] with column b all
            # ones -> the matmul routes batch b's partition-sum into PSUM row
            # b, accumulating all batches into ONE psum tile (single ctx DMA).
            wz = small.tile([P, 2 * BL], F32)
            nc.vector.memset(wz[:], 0.0)
            nc.vector.memset(wz[:, BL : BL + 1], 1.0)

            acc = psum.tile([BL, D], F32, tag="acc")

            # context[b] = sum_t values[b, t, :]
            for b in range(BL):
                src = vals.ap()[b].rearrange("(p n) d -> p (n d)", p=P)  # [128, 8192]
                g = big.tile([P, NCH * D], F32, tag="giant")
                half = NCH * D // 2
                nc.sync.dma_start(out=g[:, :half], in_=src[:, :half])
                nc.sync.dma_start(out=g[:, half:], in_=src[:, half:])

                # fold 8 column chunks -> 1, in place inside g.  The first two
                # folds each read data written by exactly ONE dma_start (TRN2
                # compute instructions carry at most one sync-wait); the rest
                # are same-engine and need no cross-engine waits.
                nc.vector.tensor_add(
                    out=g[:, :2 * D], in0=g[:, :2 * D], in1=g[:, 2 * D : 4 * D]
                )
                nc.vector.tensor_add(
                    out=g[:, 4 * D : 6 * D], in0=g[:, 4 * D : 6 * D], in1=g[:, 6 * D :]
                )
                nc.vector.tensor_add(
                    out=g[:, :2 * D], in0=g[:, :2 * D], in1=g[:, 4 * D : 6 * D]
                )
                # last fold lands in a fresh tile so its ONLY writer is the DVE
                # (the PE matmul below can then carry a single sync-wait)
                a1 = mid.tile([P, D], F32, tag="a1")
                nc.vector.tensor_add(out=a1[:], in0=g[:, :D], in1=g[:, D : 2 * D])

                # fold 128 partitions -> PSUM row b on the PE
                lhsT = wz[:, BL - b : 2 * BL - b]
                nc.tensor.matmul(
                    acc[:, 0:512], lhsT, a1[:, 0:512],
                    start=(b == 0), stop=(b == BL - 1),
                )
                nc.tensor.matmul(
                    acc[:, 512:1024], lhsT, a1[:, 512:1024],
                    start=(b == 0), stop=(b == BL - 1),
                )

            ctx_sb = small.tile([BL, D], F32)
            nc.scalar.copy(out=ctx_sb[:], in_=acc[:])
            nc.gpsimd.dma_start(out=ctx_o.ap(), in_=ctx_sb[:])

    # legalizes multi-wait instructions (TRN2: max 1 sync-wait/instruction),
    # allocates registers, fuses nops
    nc.compile()
    return nc


def _get_nc():
    global _nc
    if _nc is None:
        _nc = _build()
    return _nc


def kernel(**inputs) -> tuple:
    values = np.ascontiguousarray(np.asarray(inputs["values"], dtype=np.float32))
    assert values.shape == (B, T, D), values.shape

    nc = _get_nc()
    in_maps = [
        {"values": np.ascontiguousarray(values[c * BL : (c + 1) * BL])}
        for c in range(N_CORES)
    ]
    res = run_bass_kernel_spmd(nc, in_maps, core_ids=list(range(N_CORES)))

    ctx = np.concatenate([r["ctx"] for r in res.results], axis=0)
    aw = np.concatenate([r["aw"] for r in res.results], axis=0)
    cov = np.concatenate([r["cov"] for r in res.results], axis=0)
    return ctx, aw, cov


# revision 15
# speedup vs baseline: 1.0320x; 1.0320x over previous
"""Bahdanau-attention-with-coverage kernel for Trainium2 (Bass/Tile).

The reference module applies softmax over a size-1 axis, so every attention
weight is exactly 1.0. The whole network therefore collapses:
  context_vector     = values.sum(axis=1)               # [B, D]
  attention_weights  = ones([B, T, 1])
  coverage_vector    = arange(T) broadcast to [B, T, 1]
The kernel computes exactly that, data-parallel over batch across 8 cores.
Per core: sum-over-T of a [4, 1024, 1024] f32 slab -> DMA-bound (~16MB HBM
read at ~350 GB/s/core), with the fold work hidden under the DMA window:
  - 4x 1MB input DMAs per batch, alternating the two HWDGE queues (SP/ACT)
  - per-DMA-chunk halves-fold on GpSimd (first two) / DVE (last two)
  - accumulate chain on DVE, 128->1 partition fold on the PE into one PSUM
    tile (indicator-window weights route batch b to PSUM row b)
  - trivial outputs: memset(1.0) and gpsimd iota, via SWDGE
"""

import numpy as np

import concourse.bacc as bacc
import concourse.mybir as mybir
from concourse.tile import TileContext
from concourse.bass_utils import run_bass_kernel_spmd

B, T, D = 32, 1024, 1024
N_CORES = 8
BL = B // N_CORES  # batches per core
F32 = mybir.dt.float32
P = 128            # SBUF partitions
NCH = T // P       # 8 t-rows per partition

_nc = None


def _build():
    # TRN2 codegen allows only ONE sync-wait per instruction; the structure
    # below keeps every compute/DMA instruction at a single cross-engine
    # dependency (Bacc's event-semaphore pass legalizes the rest).
    nc = bacc.Bacc("TRN2", target_bir_lowering=False, num_swdge_queues=4)

    vals = nc.dram_tensor("values", [BL, T, D], F32, kind="ExternalInput")
    ctx_o = nc.dram_tensor("ctx", [BL, D], F32, kind="ExternalOutput")
    aw_o = nc.dram_tensor("aw", [BL, T, 1], F32, kind="ExternalOutput")
    cov_o = nc.dram_tensor("cov", [BL, T, 1], F32, kind="ExternalOutput")

    with TileContext(nc) as tc:
        with (
            tc.tile_pool(name="big", bufs=4) as big,
            tc.tile_pool(name="mid", bufs=2) as mid,
            tc.tile_pool(name="small", bufs=1) as small,
            tc.tile_pool(name="psum", bufs=1, space="PSUM") as psum,
        ):
            # Sliding-window indicator weights: wz[:, BL] == 1, else 0.  For
            # batch b, lhsT = wz[:, BL-b : 2*BL-b] is [128, BL] with column b
            # all ones -> the matmul routes batch b's partition-sum into PSUM
            # row b, accumulating all batches into ONE psum tile.
            wz = small.tile([P, 2 * BL], F32)
            nc.vector.memset(wz[:], 0.0)
            nc.vector.memset(wz[:, BL : BL + 1], 1.0)

            acc = psum.tile([BL, D], F32, tag="acc")

            # context[b] = sum_t values[b, t, :]
            dma_eng = [nc.sync, nc.scalar, nc.sync, nc.scalar]
            fold_eng = [nc.gpsimd, nc.gpsimd, nc.vector, nc.vector]
            C = 2 * D  # bytes per input DMA chunk: [128, 2048] f32 = 1 MB
            for b in range(BL):
                src = vals.ap()[b].rearrange("(p n) d -> p (n d)", p=P)  # [128, 8192]
                g = big.tile([P, NCH * D], F32, tag="giant")
                for i in range(4):
                    dma_eng[i].dma_start(
                        out=g[:, i * C : (i + 1) * C], in_=src[:, i * C : (i + 1) * C]
                    )

                # per-chunk halves-fold; each op reads data from exactly ONE
                # dma_start (single sync-wait)
                f = []
                for i in range(4):
                    fi = mid.tile([P, D], F32, tag=f"f{i}", name=f"f{i}_{b}")
                    fold_eng[i].tensor_add(
                        out=fi[:],
                        in0=g[:, i * C : i * C + D],
                        in1=g[:, i * C + D : (i + 1) * C],
                    )
                    f.append(fi)

                # accumulate chain on DVE (chain slots rotate within one tag)
                c01 = mid.tile([P, D], F32, tag="chain", bufs=3, name=f"c01_{b}")
                nc.vector.tensor_add(out=c01[:], in0=f[0][:], in1=f[1][:])
                c012 = mid.tile([P, D], F32, tag="chain", bufs=3, name=f"c012_{b}")
                nc.vector.tensor_add(out=c012[:], in0=c01[:], in1=f[2][:])
                a1 = mid.tile([P, D], F32, tag="chain", bufs=3, name=f"a1_{b}")
                nc.vector.tensor_add(out=a1[:], in0=c012[:], in1=f[3][:])

                # fold 128 partitions -> PSUM row b on the PE
                lhsT = wz[:, BL - b : 2 * BL - b]
                nc.tensor.matmul(
                    acc[:, 0:512], lhsT, a1[:, 0:512],
                    start=(b == 0), stop=(b == BL - 1),
                )
                nc.tensor.matmul(
                    acc[:, 512:1024], lhsT, a1[:, 512:1024],
                    start=(b == 0), stop=(b == BL - 1),
                )

            ctx_sb = small.tile([BL, D], F32)
            nc.vector.tensor_copy(out=ctx_sb[:], in_=acc[:])
            nc.gpsimd.dma_start(out=ctx_o.ap(), in_=ctx_sb[:])

            # attention_weights = 1.0 everywhere
            ones_t = small.tile([P, BL * T // P], F32)
            nc.vector.memset(ones_t[:], 1.0)
            nc.gpsimd.dma_start(
                out=aw_o.ap().flatten_outer_dims().rearrange("(p n) c -> p (n c)", p=P),
                in_=ones_t[:],
            )

            # coverage[b, t] = t;  layout t = NCH*p + n  ->  iota = NCH*p + n
            iota_t = small.tile([P, BL, NCH], mybir.dt.int32)
            nc.gpsimd.iota(
                iota_t[:], pattern=[[0, BL], [1, NCH]], base=0, channel_multiplier=NCH
            )
            cov_t = small.tile([P, BL, NCH], F32)
            nc.vector.tensor_copy(out=cov_t[:], in_=iota_t[:])
            nc.gpsimd.dma_start(
                out=cov_o.ap().rearrange("b (p n) c -> p b (n c)", p=P),
                in_=cov_t[:],
            )

    # legalizes multi-wait instructions (TRN2: max 1 sync-wait/instruction),
    # allocates registers, fuses nops
    nc.compile()
    return nc


def _get_nc():
    global _nc
    if _nc is None:
        _nc = _build()
    return _nc


def kernel(**inputs) -> tuple:
    values = np.ascontiguousarray(np.asarray(inputs["values"], dtype=np.float32))
    assert values.shape == (B, T, D), values.shape

    nc = _get_nc()
    in_maps = [
        {"values": np.ascontiguousarray(values[c * BL : (c + 1) * BL])}
        for c in range(N_CORES)
    ]
    res = run_bass_kernel_spmd(nc, in_maps, core_ids=list(range(N_CORES)))

    ctx = np.concatenate([r["ctx"] for r in res.results], axis=0)
    aw = np.concatenate([r["aw"] for r in res.results], axis=0)
    cov = np.concatenate([r["cov"] for r in res.results], axis=0)
    return ctx, aw, cov
